# revision 8
# baseline (speedup 1.0000x reference)
"""MultiLabelMarginLoss kernel for Trainium2, 8 cores data-parallel.

Reference (B=64, C=1536):
    loss = mean_i [ sum_{j in pos_i, n in neg_i} relu(1 - x_j + x_n) / (k_i (C-k_i)) ]

Algorithm (per core, 8 samples assigned by a host-side load balancer):
  All valid positives of the core's 8 samples are packed into NB blocks of
  128 "slots". For block b, a fp8 DoubleRow selector matmul broadcasts each
  slot's sample row [pred row (1536) | xm section (128 = own positives,
  padded)] into PSUM — hi+lo fp8 split keeps the broadcast within ~0.4%.
  Three engines then consume disjoint column slices of the block with a
  fused elementwise+accumulate instruction each:
    ScalarE : relu(x + b_q) via activation(Relu, bias)        -> accA
    VectorE : sum_c max(x, t_q) via tensor_scalar(max, add)   -> accD
    Pool    : same                                            -> accP
  with t_q = xp_q - 1, b_q = -t_q.  Identity used host-side (exact, works
  for pads too):   sum_slice relu(b + v) = acc_slice + n_slice * b.
  The xm section gives the positive-vs-positive correction (subtracted on
  host).  Host finishes in fp64: loss = sum_q sigma_s [R_main - R_xm].

Everything data-dependent (selectors, t/b columns, hi/lo rows) is prepared
host-side; the device only runs matmuls + fused consumers + 2 in / 1 out DMA.
"""

import numpy as np
from contextlib import ExitStack

import ml_dtypes

import concourse.bass as bass
import concourse.tile as tile
from concourse import bacc, mybir
from concourse.bass_utils import run_bass_kernel_spmd

B, C = 64, 1536
M = 8                 # cores
BL = B // M           # samples per core
XMW = 128             # xm (correction) section width
ROWW = C + XMW        # logical row width per sample: pred | xm
PADV = -192.0         # xm pad value (way below any t = xp-1)
FP32 = mybir.dt.float32
FP8 = mybir.dt.float8e4
F8NP = ml_dtypes.float8_e4m3
RELU = mybir.ActivationFunctionType.Relu
MAXOP = mybir.AluOpType.max
ADDOP = mybir.AluOpType.add
DR = mybir.MatmulPerfMode.DoubleRow

# per-block column slices: [0:1024) ACT (2-bank tile, 1 instr),
# [1024:1536) DVE, xm [1536:1664) DVE.  (GPSIMD cannot read PSUM, so the
# Pool engine cannot consume broadcast blocks at all.)
SA = (0, 1024)
SD = (1024, 1536)


def _xm_eng(b, nb):
    return "D"


def _build_nc(nb):
    W8 = 2 * ROWW + 2 * 128 * nb      # fp8 cols in rows blob
    nxd = nb
    nacc = 3 * nb                     # accA nb + accD nb+nxd

    nc = bacc.Bacc("TRN2", target_bir_lowering=False, debug=False, num_devices=M)
    rows_d = nc.dram_tensor("rows", [BL, W8], FP8, kind="ExternalInput")
    tb_d = nc.dram_tensor("tb", [128, 2 * nb], FP32, kind="ExternalInput")
    out_d = nc.dram_tensor("out", [128, nacc], FP32, kind="ExternalOutput")

    with tile.TileContext(nc) as tc, ExitStack() as ctx:
        sbuf = ctx.enter_context(tc.tile_pool(name="sbuf", bufs=1))
        psum = ctx.enter_context(tc.tile_pool(name="psum", bufs=2, space="PSUM"))

        # ACT table preload overlapped with input DMAs
        wt = sbuf.tile([128, 1], FP32)
        nc.vector.memset(wt[:], 0.0)
        wo = sbuf.tile([128, 1], FP32)
        nc.scalar.activation(wo[:], wt[:], RELU)

        rows = sbuf.tile([BL, W8], FP8)
        nc.sync.dma_start(rows[:], rows_d.ap())
        tb = sbuf.tile([128, 2 * nb], FP32)
        nc.gpsimd.dma_start(tb[:], tb_d.ap())

        accA = sbuf.tile([128, nb], FP32)
        accD = sbuf.tile([128, nb + nxd], FP32)
        scrA = sbuf.tile([128, 1024], FP32)

        hiv = rows[:, : 2 * ROWW].rearrange("p (i n) -> p i n", i=2)

        di = pi = 0
        for b in range(nb):
            selv = rows[:, 2 * ROWW + b * 256: 2 * ROWW + (b + 1) * 256] \
                .rearrange("p (i q) -> p i q", i=2)
            tcol = tb[:, b: b + 1]
            bcol = tb[:, nb + b: nb + b + 1]

            bA = psum.tile([128, 1024], FP32, tag="bA")
            bD = psum.tile([128, 512], FP32, tag="bD")
            bX = psum.tile([128, XMW], FP32, tag="bX")

            for lo, hi, t in ((SA[0], SA[1], bA), (SD[0], SD[1], bD)):
                for c0 in range(lo, hi, 256):
                    nc.tensor.matmul(
                        t[:, c0 - lo: c0 - lo + 256], lhsT=selv,
                        rhs=hiv[:, :, c0: c0 + 256],
                        start=True, stop=True, perf_mode=DR,
                    )
            nc.tensor.matmul(bX[:], lhsT=selv, rhs=hiv[:, :, C: C + XMW],
                             start=True, stop=True, perf_mode=DR)

            nc.scalar.activation(scrA[:], bA[:], RELU, bias=bcol, scale=1.0,
                                 accum_out=accA[:, b: b + 1])
            nc.vector.tensor_scalar(bD[:], bD[:], tcol, None, op0=MAXOP,
                                    op1=ADDOP, accum_out=accD[:, b: b + 1])
            nc.vector.tensor_scalar(bX[:], bX[:], tcol, None, op0=MAXOP,
                                    op1=ADDOP,
                                    accum_out=accD[:, nb + di: nb + di + 1])
            di += 1

        out_t = sbuf.tile([128, nacc], FP32)
        nc.vector.tensor_copy(out_t[:, :nb], accA[:])
        nc.vector.tensor_copy(out_t[:, nb:], accD[:])
        nc.sync.dma_start(out_d.ap(), out_t[:])

    nc.compile()
    return nc


_NC_CACHE = {}


def _get_nc(nb):
    if nb not in _NC_CACHE:
        _NC_CACHE[nb] = _build_nc(nb)
    return _NC_CACHE[nb]


def _hi_lo(x):
    hi = x.astype(F8NP)
    lo = (x - hi.astype(np.float32)).astype(F8NP)
    return hi, lo


def prepare(pred, target):
    """Host prep: returns (nb, in_maps, postprocess_fn)."""
    pred = np.ascontiguousarray(np.asarray(pred), dtype=np.float32)
    tgt = np.asarray(target)
    assert pred.shape == (B, C) and tgt.shape == (B, C)

    # --- host: valid positives per sample ---
    valid = np.cumprod(tgt != -1, axis=1).astype(bool)
    ks = valid.sum(axis=1).astype(np.int64)          # [B]
    # --- load-balance samples across cores (greedy, 8 per core) ---
    order = np.argsort(-ks, kind="stable")
    core_of = np.empty(B, dtype=np.int64)
    sums = np.zeros(M, dtype=np.int64)
    counts = np.zeros(M, dtype=np.int64)
    for s in order:
        c = min((c for c in range(M) if counts[c] < BL), key=lambda c: sums[c])
        core_of[s] = c
        sums[c] += ks[s]
        counts[c] += 1
    nb = int(max(1, -(-int(sums.max()) // 128)))
    nxd = nb

    W8 = 2 * ROWW + 2 * 128 * nb
    in_maps = []
    meta = []                       # per core: list of (s_global, t, b) per slot
    for c in range(M):
        samp = [int(s) for s in range(B) if core_of[s] == c]
        rows8 = np.zeros((BL, ROWW), dtype=np.float32)
        slot_meta = []              # (sample_local, t_q, b_q)
        sels = np.zeros((BL, nb * 128), dtype=np.float32)
        tcols = np.zeros((128, nb), dtype=np.float32)
        bcols = np.zeros((128, nb), dtype=np.float32)
        pos = 0
        for sl, s in enumerate(samp):
            k = int(ks[s])
            idx = tgt[s, :k].astype(np.int64)
            xp = pred[s, idx]
            rows8[sl, :C] = pred[s]
            rows8[sl, C:] = PADV
            rows8[sl, C: C + k] = xp
            for j in range(k):
                q, bq = pos % 128, pos // 128
                t = float(xp[j]) - 1.0
                sels[sl, bq * 128 + q] = 1.0
                tcols[q, bq] = t
                bcols[q, bq] = -t
                slot_meta.append((s, sl, t))
                pos += 1
        hi, lo = _hi_lo(rows8)
        blob = np.zeros((BL, W8), dtype=F8NP)
        blob[:, 0: ROWW] = hi
        blob[:, ROWW: 2 * ROWW] = lo
        selq = sels.reshape(BL, nb, 128)
        for bq in range(nb):
            base = 2 * ROWW + bq * 256
            blob[:, base: base + 128] = selq[:, bq, :].astype(F8NP)
            blob[:, base + 128: base + 256] = selq[:, bq, :].astype(F8NP)
        tbm = np.concatenate([tcols, bcols], axis=1).astype(np.float32)
        in_maps.append({"rows": blob, "tb": tbm})
        meta.append((samp, slot_meta, hi, lo))

    def post(outs):
        """outs: list of M arrays [128, nacc] -> fp64 loss."""
        nD = SD[1] - SD[0]
        total = 0.0
        for c in range(M):
            samp, slot_meta, hi, lo = meta[c]
            out = np.asarray(outs[c], dtype=np.float64)
            accA = out[:, :nb]
            accD = out[:, nb:]
            for i, (s, sl, t) in enumerate(slot_meta):
                q, bq = i % 128, i // 128
                bqv = -t
                r_main = accA[q, bq] + (accD[q, bq] + nD * bqv)
                r_xm = accD[q, nb + bq] + XMW * bqv
                k = int(ks[s])
                sigma = 1.0 / (k * (C - k) * B)
                total += sigma * (r_main - r_xm)
        return total

    return nb, in_maps, post


def kernel(pred, target):
    nb, in_maps, post = prepare(pred, target)
    nc = _get_nc(nb)
    res = run_bass_kernel_spmd(nc, in_maps, core_ids=list(range(M)))
    total = post([np.asarray(r["out"]) for r in res.results])
    return np.asarray(total, dtype=np.float32)


# revision 10
# speedup vs baseline: 1.0810x; 1.0810x over previous
"""MultiLabelMarginLoss kernel for Trainium2, 8 cores data-parallel.

Reference (B=64, C=1536):
    loss = mean_i [ sum_{j in pos_i, n in neg_i} relu(1 - x_j + x_n) / (k_i (C-k_i)) ]

Algorithm (per core, 8 samples assigned by a host-side load balancer):
  All valid positives of the core's 8 samples are packed into NB blocks of
  128 "slots".  Each sample's pred row is shipped with its OWN positive
  columns masked to -192, so positive-vs-positive pairs vanish inside the
  main reduction and no separate correction pass is needed (relu(-192+b)=0
  exactly, and the max-identity compensation below is exact for them too).

  For block b, a fp8 DoubleRow selector matmul broadcasts each slot's
  masked sample row into PSUM (hi+lo fp8 split keeps the broadcast within
  ~0.4%).  Two engines then consume disjoint column slices, each with one
  fused elementwise+accumulate instruction per block:
    ScalarE : relu(x + b_q)        via activation(Relu, bias)      -> accA
    VectorE : sum_c max(x, t_q)    via tensor_scalar(max, add-red) -> accD
  with t_q = xp_q - 1, b_q = -t_q.  Host-side identity (exact, pads too):
      sum_slice relu(b + v) = acc_slice + n_slice * b.
  Host finishes in fp64: loss = sum_q sigma_s(q) * R_q.

Everything data-dependent (selectors, t/b columns, hi/lo rows) is prepared
host-side; the device runs 6 matmuls + 2 consumer instrs per block and
2 in / 1 out DMAs.
"""

import numpy as np
from contextlib import ExitStack

import ml_dtypes

import concourse.bass as bass
import concourse.tile as tile
from concourse import bacc, mybir
from concourse.bass_utils import run_bass_kernel_spmd

B, C = 64, 1536
M = 8                 # cores
BL = B // M           # samples per core
MASKV = -192.0        # positive-column mask (exact in fp8; << min(t))
FP32 = mybir.dt.float32
FP8 = mybir.dt.float8e4
F8NP = ml_dtypes.float8_e4m3
RELU = mybir.ActivationFunctionType.Relu
MAXOP = mybir.AluOpType.max
ADDOP = mybir.AluOpType.add
DR = mybir.MatmulPerfMode.DoubleRow

# per-block column split: [0:SA) ACT, [SA:C) DVE; both tiles are 2 PSUM
# banks, x2 bufs = 8 banks total.
SA = 720
SD = C - SA           # 816


def _build_nc(nb):
    W8 = 2 * C + 2 * 128 * nb         # fp8 cols in rows blob (hi|lo planes + sel)
    nacc = 2 * nb

    nc = bacc.Bacc("TRN2", target_bir_lowering=False, debug=False, num_devices=M)
    rows_d = nc.dram_tensor("rows", [BL, W8], FP8, kind="ExternalInput")
    tb_d = nc.dram_tensor("tb", [128, 2 * nb], FP32, kind="ExternalInput")
    out_d = nc.dram_tensor("out", [128, nacc], FP32, kind="ExternalOutput")

    with tile.TileContext(nc) as tc, ExitStack() as ctx:
        sbuf = ctx.enter_context(tc.tile_pool(name="sbuf", bufs=1))
        psum = ctx.enter_context(tc.tile_pool(name="psum", bufs=2, space="PSUM"))

        rows = sbuf.tile([BL, W8], FP8)
        nc.sync.dma_start(rows[:], rows_d.ap())
        tb = sbuf.tile([128, 2 * nb], FP32)
        nc.gpsimd.dma_start(tb[:], tb_d.ap())

        # ACT table preload overlapped with input DMAs
        wt = sbuf.tile([128, 1], FP32)
        nc.vector.memset(wt[:], 0.0)
        wo = sbuf.tile([128, 1], FP32)
        nc.scalar.activation(wo[:], wt[:], RELU)

        accA = sbuf.tile([128, nb], FP32)
        accD = sbuf.tile([128, nb], FP32)
        scrA = sbuf.tile([128, SA], FP32)

        hiv = rows[:, : 2 * C].rearrange("p (i n) -> p i n", i=2)

        for b in range(nb):
            selv = rows[:, 2 * C + b * 256: 2 * C + (b + 1) * 256] \
                .rearrange("p (i q) -> p i q", i=2)
            tcol = tb[:, b: b + 1]
            bcol = tb[:, nb + b: nb + b + 1]

            bA = psum.tile([128, SA], FP32, tag="bA")
            bD = psum.tile([128, SD], FP32, tag="bD")

            def chunks(width):
                # <=256-col pieces that never cross a 512-col PSUM bank line
                o = 0
                while o < width:
                    w = min(256, width - o, 512 - (o % 512) or 512)
                    yield o, w
                    o += w

            for o, w in chunks(SA):
                nc.tensor.matmul(bA[:, o: o + w], lhsT=selv,
                                 rhs=hiv[:, :, o: o + w],
                                 start=True, stop=True, perf_mode=DR)
            for o, w in chunks(SD):
                nc.tensor.matmul(bD[:, o: o + w], lhsT=selv,
                                 rhs=hiv[:, :, SA + o: SA + o + w],
                                 start=True, stop=True, perf_mode=DR)

            nc.scalar.activation(scrA[:], bA[:], RELU, bias=bcol, scale=1.0,
                                 accum_out=accA[:, b: b + 1])
            nc.vector.tensor_scalar(bD[:], bD[:], tcol, None, op0=MAXOP,
                                    op1=ADDOP, accum_out=accD[:, b: b + 1])

        out_t = sbuf.tile([128, nacc], FP32)
        nc.vector.tensor_copy(out_t[:, :nb], accA[:])
        nc.vector.tensor_copy(out_t[:, nb:], accD[:])
        nc.sync.dma_start(out_d.ap(), out_t[:])

    nc.compile()
    return nc


_NC_CACHE = {}


def _get_nc(nb):
    if nb not in _NC_CACHE:
        _NC_CACHE[nb] = _build_nc(nb)
    return _NC_CACHE[nb]


def _hi_lo(x):
    hi = x.astype(F8NP)
    lo = (x - hi.astype(np.float32)).astype(F8NP)
    return hi, lo


def prepare(pred, target):
    """Host prep: returns (nb, in_maps, postprocess_fn)."""
    pred = np.ascontiguousarray(np.asarray(pred), dtype=np.float32)
    tgt = np.asarray(target)
    assert pred.shape == (B, C) and tgt.shape == (B, C)

    # valid positives per sample
    valid = np.cumprod(tgt != -1, axis=1).astype(bool)
    ks = valid.sum(axis=1).astype(np.int64)          # [B]
    # load-balance samples across cores (greedy, 8 per core)
    order = np.argsort(-ks, kind="stable")
    core_of = np.empty(B, dtype=np.int64)
    sums = np.zeros(M, dtype=np.int64)
    counts = np.zeros(M, dtype=np.int64)
    for s in order:
        c = min((c for c in range(M) if counts[c] < BL), key=lambda c: sums[c])
        core_of[s] = c
        sums[c] += ks[s]
        counts[c] += 1
    nb = int(max(1, -(-int(sums.max()) // 128)))

    W8 = 2 * C + 2 * 128 * nb
    in_maps = []
    meta = []
    for c in range(M):
        samp = [int(s) for s in range(B) if core_of[s] == c]
        rows8 = np.zeros((BL, C), dtype=np.float32)
        slot_meta = []              # (sample_global, t_q)
        sels = np.zeros((BL, nb * 128), dtype=np.float32)
        tcols = np.zeros((128, nb), dtype=np.float32)
        bcols = np.zeros((128, nb), dtype=np.float32)
        pos = 0
        for sl, s in enumerate(samp):
            k = int(ks[s])
            idx = tgt[s, :k].astype(np.int64)
            xp = pred[s, idx]
            rows8[sl] = pred[s]
            rows8[sl, idx] = MASKV          # kill pos-vs-pos pairs
            for j in range(k):
                q, bq = pos % 128, pos // 128
                t = float(xp[j]) - 1.0
                sels[sl, bq * 128 + q] = 1.0
                tcols[q, bq] = t
                bcols[q, bq] = -t
                slot_meta.append((s, t))
                pos += 1
        hi, lo = _hi_lo(rows8)
        blob = np.zeros((BL, W8), dtype=F8NP)
        blob[:, 0: C] = hi
        blob[:, C: 2 * C] = lo
        selq = sels.reshape(BL, nb, 128)
        for bq in range(nb):
            base = 2 * C + bq * 256
            blob[:, base: base + 128] = selq[:, bq, :].astype(F8NP)
            blob[:, base + 128: base + 256] = selq[:, bq, :].astype(F8NP)
        tbm = np.concatenate([tcols, bcols], axis=1).astype(np.float32)
        in_maps.append({"rows": blob, "tb": tbm})
        meta.append(slot_meta)

    def post(outs):
        """outs: list of M arrays [128, 2*nb] -> fp64 loss."""
        total = 0.0
        for c in range(M):
            out = np.asarray(outs[c], dtype=np.float64)
            accA = out[:, :nb]
            accD = out[:, nb:]
            for i, (s, t) in enumerate(meta[c]):
                q, bq = i % 128, i // 128
                r = accA[q, bq] + (accD[q, bq] + SD * (-t))
                k = int(ks[s])
                total += r / (k * (C - k) * B)
        return total

    return nb, in_maps, post


def kernel(pred, target):
    nb, in_maps, post = prepare(pred, target)
    nc = _get_nc(nb)
    res = run_bass_kernel_spmd(nc, in_maps, core_ids=list(range(M)))
    total = post([np.asarray(r["out"]) for r in res.results])
    return np.asarray(total, dtype=np.float32)


# revision 11
# speedup vs baseline: 1.1023x; 1.0197x over previous
"""MultiLabelMarginLoss kernel for Trainium2, 8 cores data-parallel.

Reference (B=64, C=1536):
    loss = mean_i [ sum_{j in pos_i, n in neg_i} relu(1 - x_j + x_n) / (k_i (C-k_i)) ]

Algorithm (per core, 8 samples assigned by a host-side load balancer):
  All valid positives of the core's 8 samples are packed into NB blocks of
  128 "slots".  Each sample's pred row is shipped with its OWN positive
  columns masked to -192, so positive-vs-positive pairs vanish inside the
  main reduction and no separate correction pass is needed (relu(-192+b)=0
  exactly, and the max-identity compensation below is exact for them too).

  For block b, a fp8 DoubleRow selector matmul broadcasts each slot's
  masked sample row into PSUM (hi+lo fp8 split keeps the broadcast within
  ~0.4%).  Two engines then consume disjoint column slices, each with one
  fused elementwise+accumulate instruction per block:
    ScalarE : relu(x + b_q)        via activation(Relu, bias)      -> accA
    VectorE : sum_c max(x, t_q)    via tensor_scalar(max, add-red) -> accD
  with t_q = xp_q - 1, b_q = -t_q.  Host-side identity (exact, pads too):
      sum_slice relu(b + v) = acc_slice + n_slice * b.
  Host finishes in fp64: loss = sum_q sigma_s(q) * R_q.

Everything data-dependent (selectors, t/b columns, hi/lo rows) is prepared
host-side; the device runs 6 matmuls + 2 consumer instrs per block and
2 in / 1 out DMAs.
"""

import numpy as np
from contextlib import ExitStack

import ml_dtypes

import concourse.bass as bass
import concourse.tile as tile
from concourse import bacc, mybir
from concourse.bass_utils import run_bass_kernel_spmd

B, C = 64, 1536
M = 8                 # cores
BL = B // M           # samples per core
MASKV = -192.0        # positive-column mask (exact in fp8; << min(t))
FP32 = mybir.dt.float32
FP8 = mybir.dt.float8e4
F8NP = ml_dtypes.float8_e4m3
RELU = mybir.ActivationFunctionType.Relu
MAXOP = mybir.AluOpType.max
ADDOP = mybir.AluOpType.add
DR = mybir.MatmulPerfMode.DoubleRow

# per-block column split: [0:SA) ACT, [SA:C) DVE; both tiles are 2 PSUM
# banks, x2 bufs = 8 banks total.
SA = 684
SD = C - SA           # 852


def _build_nc(nb):
    W8 = 2 * C + 2 * 128 * nb         # fp8 cols in rows blob (hi|lo planes + sel)
    nacc = 2 * nb

    nc = bacc.Bacc("TRN2", target_bir_lowering=False, debug=False, num_devices=M)
    rows_d = nc.dram_tensor("rows", [BL, W8], FP8, kind="ExternalInput")
    tb_d = nc.dram_tensor("tb", [128, 2 * nb], FP32, kind="ExternalInput")
    out_d = nc.dram_tensor("out", [128, nacc], FP32, kind="ExternalOutput")

    with tile.TileContext(nc) as tc, ExitStack() as ctx:
        sbuf = ctx.enter_context(tc.tile_pool(name="sbuf", bufs=1))
        psum = ctx.enter_context(tc.tile_pool(name="psum", bufs=2, space="PSUM"))

        tb = sbuf.tile([128, 2 * nb], FP32)
        nc.gpsimd.dma_start(tb[:], tb_d.ap())
        rows = sbuf.tile([BL, W8], FP8)
        nc.sync.dma_start(rows[:], rows_d.ap())

        # ACT table preload overlapped with input DMAs
        wt = sbuf.tile([128, 1], FP32)
        nc.vector.memset(wt[:], 0.0)
        wo = sbuf.tile([128, 1], FP32)
        nc.scalar.activation(wo[:], wt[:], RELU)

        acc = sbuf.tile([128, nacc], FP32)
        scrA = sbuf.tile([128, SA], FP32)

        hiv = rows[:, : 2 * C].rearrange("p (i n) -> p i n", i=2)

        for b in range(nb):
            selv = rows[:, 2 * C + b * 256: 2 * C + (b + 1) * 256] \
                .rearrange("p (i q) -> p i q", i=2)
            tcol = tb[:, b: b + 1]
            bcol = tb[:, nb + b: nb + b + 1]

            bA = psum.tile([128, SA], FP32, tag="bA")
            bD = psum.tile([128, SD], FP32, tag="bD")

            def chunks(width):
                # <=256-col pieces that never cross a 512-col PSUM bank line
                o = 0
                while o < width:
                    w = min(256, width - o, 512 - (o % 512) or 512)
                    yield o, w
                    o += w

            for o, w in chunks(SD):
                nc.tensor.matmul(bD[:, o: o + w], lhsT=selv,
                                 rhs=hiv[:, :, SA + o: SA + o + w],
                                 start=True, stop=True, perf_mode=DR)
            for o, w in chunks(SA):
                nc.tensor.matmul(bA[:, o: o + w], lhsT=selv,
                                 rhs=hiv[:, :, o: o + w],
                                 start=True, stop=True, perf_mode=DR)

            nc.vector.tensor_scalar(bD[:], bD[:], tcol, None, op0=MAXOP,
                                    op1=ADDOP, accum_out=acc[:, nb + b: nb + b + 1])
            nc.scalar.activation(scrA[:], bA[:], RELU, bias=bcol, scale=1.0,
                                 accum_out=acc[:, b: b + 1])

        nc.sync.dma_start(out_d.ap(), acc[:])

    nc.compile()
    return nc


_NC_CACHE = {}


def _get_nc(nb):
    if nb not in _NC_CACHE:
        _NC_CACHE[nb] = _build_nc(nb)
    return _NC_CACHE[nb]


def _hi_lo(x):
    hi = x.astype(F8NP)
    lo = (x - hi.astype(np.float32)).astype(F8NP)
    return hi, lo


def prepare(pred, target):
    """Host prep: returns (nb, in_maps, postprocess_fn)."""
    pred = np.ascontiguousarray(np.asarray(pred), dtype=np.float32)
    tgt = np.asarray(target)
    assert pred.shape == (B, C) and tgt.shape == (B, C)

    # valid positives per sample
    valid = np.cumprod(tgt != -1, axis=1).astype(bool)
    ks = valid.sum(axis=1).astype(np.int64)          # [B]
    # load-balance samples across cores (greedy, 8 per core)
    order = np.argsort(-ks, kind="stable")
    core_of = np.empty(B, dtype=np.int64)
    sums = np.zeros(M, dtype=np.int64)
    counts = np.zeros(M, dtype=np.int64)
    for s in order:
        c = min((c for c in range(M) if counts[c] < BL), key=lambda c: sums[c])
        core_of[s] = c
        sums[c] += ks[s]
        counts[c] += 1
    nb = int(max(1, -(-int(sums.max()) // 128)))

    W8 = 2 * C + 2 * 128 * nb
    in_maps = []
    meta = []
    for c in range(M):
        samp = [int(s) for s in range(B) if core_of[s] == c]
        rows8 = np.zeros((BL, C), dtype=np.float32)
        slot_meta = []              # (sample_global, t_q)
        sels = np.zeros((BL, nb * 128), dtype=np.float32)
        tcols = np.zeros((128, nb), dtype=np.float32)
        bcols = np.zeros((128, nb), dtype=np.float32)
        pos = 0
        for sl, s in enumerate(samp):
            k = int(ks[s])
            idx = tgt[s, :k].astype(np.int64)
            xp = pred[s, idx]
            rows8[sl] = pred[s]
            rows8[sl, idx] = MASKV          # kill pos-vs-pos pairs
            for j in range(k):
                q, bq = pos % 128, pos // 128
                t = float(xp[j]) - 1.0
                sels[sl, bq * 128 + q] = 1.0
                tcols[q, bq] = t
                bcols[q, bq] = -t
                slot_meta.append((s, t))
                pos += 1
        hi, lo = _hi_lo(rows8)
        blob = np.zeros((BL, W8), dtype=F8NP)
        blob[:, 0: C] = hi
        blob[:, C: 2 * C] = lo
        selq = sels.reshape(BL, nb, 128)
        for bq in range(nb):
            base = 2 * C + bq * 256
            blob[:, base: base + 128] = selq[:, bq, :].astype(F8NP)
            blob[:, base + 128: base + 256] = selq[:, bq, :].astype(F8NP)
        tbm = np.concatenate([tcols, bcols], axis=1).astype(np.float32)
        in_maps.append({"rows": blob, "tb": tbm})
        meta.append(slot_meta)

    def post(outs):
        """outs: list of M arrays [128, 2*nb] -> fp64 loss."""
        total = 0.0
        for c in range(M):
            out = np.asarray(outs[c], dtype=np.float64)
            accA = out[:, :nb]
            accD = out[:, nb:]
            for i, (s, t) in enumerate(meta[c]):
                q, bq = i % 128, i // 128
                r = accA[q, bq] + (accD[q, bq] + SD * (-t))
                k = int(ks[s])
                total += r / (k * (C - k) * B)
        return total

    return nb, in_maps, post


def kernel(pred, target):
    nb, in_maps, post = prepare(pred, target)
    nc = _get_nc(nb)
    res = run_bass_kernel_spmd(nc, in_maps, core_ids=list(range(M)))
    total = post([np.asarray(r["out"]) for r in res.results])
    return np.asarray(total, dtype=np.float32)
